# revision 21
# baseline (speedup 1.0000x reference)
"""Trainium2 Bass kernel for a dense transformer block (B=4, T=2048, C=1024, H=16).

Sharding: zero-collective. Each of the 8 cores owns (batch b, query-half h):
  core c -> b = c//2, half = c%2, query tokens = xb[half*1024 : half*1024+1024].
Per core (uniform SPMD program; all per-core variation is in the input data):
  - LN1 stats for the whole batch (bn_stats on x natural).
  - qkv computed from transposed x (xT) with LN folded in post-matmul:
      ln1(x) @ W = r .* (x @ (g.*W)) + (-mu*r) .* colsum(g.*W) + (b@ (g.*W) + b_attn)
    q/k kept transposed [d, t] for QK^T; v computed in natural [t, d] layout,
    augmented with a ones column so S@V also produces the softmax denominator.
  - full-rectangle attention (16 k-tiles per 512-query chunk) with host-supplied
    0/1 causal masks (per-core data, so the instruction stream is identical on
    all cores). P = exp(S/8) in bf16.
  - proj (local, full head dim), residual, LN2, fc+gelu, fc2, residual.
All matmuls run in bf16 with f32 PSUM accumulation; the residual stream,
softmax denominators and layernorm statistics stay f32.
"""
import sys
import math
import contextlib

for _p in ("/opt/trn_rl_repo", "/root/.axon_site/_ro/trn_rl_repo"):
    if _p not in sys.path:
        sys.path.append(_p)

import numpy as np
import ml_dtypes

import concourse.bass as bass
import concourse.bacc as bacc
import concourse.mybir as mybir
import concourse.tile as tile
from concourse.bass_utils import run_bass_kernel_spmd

F32 = mybir.dt.float32
BF16 = mybir.dt.bfloat16
AF = mybir.ActivationFunctionType
OP = mybir.AluOpType
BF = ml_dtypes.bfloat16

B, T, C, H = 4, 2048, 1024, 16
HD = C // H              # 64
EPS = 1e-5
TB = T                   # tokens per batch (2048)
TQ = T // 2              # query tokens per core (1024)
CK = C // 128            # 8 contraction k-tiles over C
NTT = TB // 128          # 16 token tiles per batch
NQT = TQ // 128          # 8 token tiles per core's queries
G = 4                    # head groups
HG = H // G              # 4 heads per group
DG = HG * HD             # 256 cols per group (per q/k/v)
FC = 4 * C               # 4096
NGT = FC // 128          # 32 fc tiles


def build_nc():
    nc = bacc.Bacc("TRN2", target_bir_lowering=False, debug=False, num_devices=8)

    dt_in = {
        # bf16 matmul operands
        "xT": ([C, TB], BF16), "qxT": ([C, TQ], BF16),
        "wqkv": ([C, 3 * C], BF16), "b1col": ([C, 1], BF16),
        "wproj": ([C, C], BF16), "wfc": ([C, FC], BF16),
        "wfc2": ([FC, C], BF16), "masks": ([16, 128, 512], BF16),
        # f32 data
        "xn": ([TB, C], F32), "xr": ([TQ, C], F32),
        "bq_col": ([3 * C, 1], F32), "bq_row": ([1, 3 * C], F32),
        "g1col": ([C, 1], F32),
        "bproj_bc": ([128, C], F32), "g2bc": ([128, C], F32),
        "b2bc": ([128, C], F32), "bfc_col": ([FC, 1], F32),
        "bfc2_bc": ([128, C], F32),
    }
    d = {k: nc.dram_tensor(k, sh, dt, kind="ExternalInput").ap()
         for k, (sh, dt) in dt_in.items()}
    out = nc.dram_tensor("out", [TQ, C], F32, kind="ExternalOutput").ap()

    with tile.TileContext(nc) as tc:
        with contextlib.ExitStack() as ctx:
            _build_body(nc, tc, ctx, d, out)
    nc.compile()
    return nc


def _build_body(nc, tc, ctx, d, out):
    pool = lambda name, bufs, **kw: ctx.enter_context(
        tc.tile_pool(name=name, bufs=bufs, **kw))

    cons = pool("cons", 1)
    small = pool("small", 2)
    stats = pool("stats", 3)
    ps = pool("ps", 4, space="PSUM")
    dram = pool("dram", 2, space="DRAM")

    # ---- constants ----
    ones_col = cons.tile([128, 1], BF16)
    nc.vector.memset(ones_col, 1.0)
    ones_t = cons.tile([128, 64], BF16)
    nc.vector.memset(ones_t, 1.0)
    eps_t = cons.tile([128, 1], F32)
    nc.vector.memset(eps_t, EPS)
    ident = cons.tile([128, 128], BF16)
    from concourse.masks import make_identity
    make_identity(nc, ident)

    g1sb = cons.tile([128, CK], F32)
    nc.sync.dma_start(out=g1sb, in_=d["g1col"].rearrange("(k p) o -> p (k o)", p=128))
    b1sb = cons.tile([128, CK], BF16)
    nc.sync.dma_start(out=b1sb, in_=d["b1col"].rearrange("(k p) o -> p (k o)", p=128))

    r_col = cons.tile([128, NTT], F32)
    mr_col = cons.tile([128, NTT], F32)
    rq_col = cons.tile([128, NQT], F32)
    mrq_col = cons.tile([128, NQT], F32)

    # ---- phase 0: LN1 stats over the full batch (and over own queries) ----
    def ln_stats(src_ap, n_tiles, rc, mrc):
        for tt in range(n_tiles):
            xt_f = stats.tile([128, C], F32, name="xt_f")
            nc.sync.dma_start(out=xt_f, in_=src_ap[tt * 128:(tt + 1) * 128, :])
            st = stats.tile([128, 2, 6], F32, name="st")
            resh = xt_f.rearrange("p (n f) -> p n f", f=512)
            for i in range(2):
                nc.vector.bn_stats(out=st[:, i, :], in_=resh[:, i, :])
            mv = stats.tile([128, 2], F32, name="mv")
            nc.vector.bn_aggr(out=mv, in_=st)
            sd = stats.tile([128, 1], F32, name="sd")
            nc.scalar.activation(sd, mv[:, 1:2], AF.Sqrt, bias=eps_t)
            nc.vector.reciprocal(rc[:, tt:tt + 1], sd)
            nc.vector.tensor_tensor(mrc[:, tt:tt + 1], mv[:, 0:1],
                                    rc[:, tt:tt + 1], op=OP.mult)
            nc.scalar.mul(mrc[:, tt:tt + 1], mrc[:, tt:tt + 1], -1.0)

    ln_stats(d["xn"], NTT, r_col, mr_col)
    ln_stats(d["xr"], NQT, rq_col, mrq_col)

    ypool = pool("ypool", 1)
    yT2 = ypool.tile([128, H // 2, TQ], BF16)

    # broadcast stats rows across partitions via a DRAM roundtrip
    attn_ctx0 = contextlib.ExitStack()
    abuf = attn_ctx0.enter_context(tc.tile_pool(name="abuf", bufs=1))
    r_bc = abuf.tile([128, TB], F32)
    mr_bc = abuf.tile([128, TB], F32)
    rq_bc = abuf.tile([128, TQ], F32)
    mrq_bc = abuf.tile([128, TQ], F32)

    def row_bcast(col_tile, n_tiles, dst):
        scr = dram.tile([n_tiles, 128], F32, name="scr")
        nc.gpsimd.dma_start(out=scr.rearrange("t p -> p t"),
                            in_=col_tile[:, 0:n_tiles])
        flat = scr.rearrange("t p -> (t p)").unsqueeze(0)
        nc.gpsimd.dma_start(out=dst, in_=flat.to_broadcast([128, n_tiles * 128]))

    row_bcast(r_col, NTT, r_bc)
    row_bcast(mr_col, NTT, mr_bc)
    row_bcast(rq_col, NQT, rq_bc)
    row_bcast(mrq_col, NQT, mrq_bc)

    # ---- attention-scope pools ----
    masksb = abuf.tile([128, 16, 512], BF16)
    nc.sync.dma_start(out=masksb, in_=d["masks"].rearrange("k p q -> p k q"))
    ones_mask = abuf.tile([128, 512], BF16)
    nc.vector.memset(ones_mask, 1.0)

    attn_ctx = contextlib.ExitStack()
    apool = lambda name, bufs, **kw: attn_ctx.enter_context(
        tc.tile_pool(name=name, bufs=bufs, **kw))
    wp = apool("wp", 1)
    xtp = apool("xtp", 2)
    qkv = apool("qkv", 1)
    pp = apool("pp", 16)
    stg = apool("stg", 2)
    qke = apool("qke", 2)
    psy = apool("psy", 3, space="PSUM")
    psb = apool("psb", 1, space="PSUM")

    qT = qkv.tile([128, HG, TQ], BF16, name="qT")
    kT = qkv.tile([128, HG, TB], BF16, name="kT")
    va = qkv.tile([128, NTT, HG, 128], BF16, name="va")
    nc.vector.memset(qT, 0.0)
    nc.vector.memset(kT, 0.0)
    nc.vector.memset(va, 0.0)
    nc.vector.memset(va[:, :, :, HD:HD + 1], 1.0)

    for g in range(G):
        # -- weights for this head group, scaled by ln1 gain --
        wt = wp.tile([128, CK, 3 * DG], BF16, name="wt")
        for kt in range(CK):
            for j, base in enumerate((0, C, 2 * C)):
                nc.sync.dma_start(
                    out=wt[:, kt, j * DG:(j + 1) * DG],
                    in_=d["wqkv"][kt * 128:(kt + 1) * 128,
                                  base + g * DG: base + (g + 1) * DG])
            nc.vector.tensor_scalar_mul(wt[:, kt, :], in0=wt[:, kt, :],
                                        scalar1=g1sb[:, kt:kt + 1])

        # -- s (colsum) and beta (b1 @ W' + b_attn) for q,k in column form --
        s_col, b_col = {}, {}
        for xi, xb in ((0, 0), (1, DG)):           # 0=q, 1=k
            for dt_ in range(2):
                ps_s = ps.tile([128, 512], F32, name="ps")
                ps_b = ps.tile([128, 512], F32, name="ps")
                for kt in range(CK):
                    w_sl = wt[:, kt, xb + dt_ * 128: xb + (dt_ + 1) * 128]
                    nc.tensor.matmul(ps_s[:, 0:1], w_sl, ones_col,
                                     start=(kt == 0), stop=(kt == CK - 1))
                    nc.tensor.matmul(ps_b[:, 0:1], w_sl, b1sb[:, kt:kt + 1],
                                     start=(kt == 0), stop=(kt == CK - 1))
                sc = small.tile([128, 1], F32, name=f"sc{xi}{dt_}")
                nc.scalar.copy(sc, ps_s[:, 0:1])
                s_col[(xi, dt_)] = sc
                bq_sl = small.tile([128, 1], F32, name=f"bq{xi}{dt_}")
                nc.sync.dma_start(
                    out=bq_sl,
                    in_=d["bq_col"][xi * C + g * DG + dt_ * 128:
                                    xi * C + g * DG + (dt_ + 1) * 128, :])
                bc = small.tile([128, 1], F32, name=f"bc{xi}{dt_}")
                nc.vector.tensor_tensor(bc, ps_b[:, 0:1], bq_sl, op=OP.add)
                b_col[(xi, dt_)] = bc

        # -- s and beta for v in row form, broadcast via DRAM roundtrip --
        ps_sv = ps.tile([128, 512], F32, name="ps")
        ps_bv = ps.tile([128, 512], F32, name="ps")
        for kt in range(CK):
            wv = wt[:, kt, 2 * DG:3 * DG]
            nc.tensor.matmul(ps_sv[0:1, 0:DG], ones_col, wv,
                             start=(kt == 0), stop=(kt == CK - 1))
            nc.tensor.matmul(ps_bv[0:1, 0:DG], b1sb[:, kt:kt + 1], wv,
                             start=(kt == 0), stop=(kt == CK - 1))
        sv_row = small.tile([1, DG], F32, name="sv_row")
        nc.scalar.copy(sv_row, ps_sv[0:1, 0:DG])
        bqv_sl = small.tile([1, DG], F32, name="bqv_sl")
        nc.sync.dma_start(out=bqv_sl,
                          in_=d["bq_row"][0:1, 2 * C + g * DG: 2 * C + (g + 1) * DG])
        bv_row = small.tile([1, DG], F32, name="bv_row")
        nc.vector.tensor_tensor(bv_row, ps_bv[0:1, 0:DG], bqv_sl, op=OP.add)
        sv_bc = small.tile([128, DG], F32, name="sv_bc")
        bv_bc = small.tile([128, DG], F32, name="bv_bc")
        for src, dst in ((sv_row, sv_bc), (bv_row, bv_bc)):
            scr2 = dram.tile([1, DG], F32, name="scr2")
            nc.gpsimd.dma_start(out=scr2, in_=src)
            nc.gpsimd.dma_start(out=dst, in_=scr2.to_broadcast([128, DG]))

        # -- qkv matmuls --
        def qk_evict(psrc, dst, dt_, sl, rbc_sl, mrbc_sl, sc, bc):
            e1 = qke.tile([128, 512], F32, name="e1")
            nc.vector.tensor_tensor(e1, psrc, rbc_sl, op=OP.mult)
            nc.vector.scalar_tensor_tensor(e1, in0=mrbc_sl, scalar=sc, in1=e1,
                                           op0=OP.mult, op1=OP.add)
            nc.vector.tensor_scalar_add(dst[0:64, 2 * dt_, sl], in0=e1[0:64, :],
                                        scalar1=bc[0:64, :])
            nc.vector.tensor_scalar_add(dst[64:128, 2 * dt_ + 1, sl],
                                        in0=e1[64:128, :], scalar1=bc[64:128, :])

        for ch in range(4):                      # k/v over the full batch
            xt = xtp.tile([128, CK, 512], BF16, name="xt")
            nc.sync.dma_start(
                out=xt,
                in_=d["xT"].rearrange("(k p) t -> p k t", p=128)[:, :, ch * 512:(ch + 1) * 512])
            for dt_ in range(2):                 # k
                psk = ps.tile([128, 512], F32, name="ps")
                for kt in range(CK):
                    nc.tensor.matmul(psk, wt[:, kt, DG + dt_ * 128:DG + (dt_ + 1) * 128],
                                     xt[:, kt, :], start=(kt == 0), stop=(kt == CK - 1))
                qk_evict(psk, kT, dt_, slice(ch * 512, (ch + 1) * 512),
                         r_bc[:, ch * 512:(ch + 1) * 512],
                         mr_bc[:, ch * 512:(ch + 1) * 512],
                         s_col[(1, dt_)], b_col[(1, dt_)])
            for tl in range(4):                  # v (natural layout)
                tt = ch * 4 + tl
                psv = ps.tile([128, 512], F32, name="ps")
                for kt in range(CK):
                    nc.tensor.matmul(psv[:, 0:DG], xt[:, kt, tl * 128:(tl + 1) * 128],
                                     wt[:, kt, 2 * DG:3 * DG],
                                     start=(kt == 0), stop=(kt == CK - 1))
                zt = qke.tile([128, DG], F32, name="zt")
                nc.vector.scalar_tensor_tensor(zt, in0=sv_bc,
                                               scalar=mr_col[:, tt:tt + 1],
                                               in1=bv_bc, op0=OP.mult, op1=OP.add)
                nc.vector.scalar_tensor_tensor(
                    va[:, tt, :, 0:HD],
                    in0=psv[:, 0:DG].rearrange("p (h d) -> p h d", h=HG),
                    scalar=r_col[:, tt:tt + 1],
                    in1=zt.rearrange("p (h d) -> p h d", h=HG),
                    op0=OP.mult, op1=OP.add)
        for ch in range(2):                      # q over own queries
            qxt = xtp.tile([128, CK, 512], BF16, name="qxt")
            nc.sync.dma_start(
                out=qxt,
                in_=d["qxT"].rearrange("(k p) t -> p k t", p=128)[:, :, ch * 512:(ch + 1) * 512])
            for dt_ in range(2):
                psq = ps.tile([128, 512], F32, name="ps")
                for kt in range(CK):
                    nc.tensor.matmul(psq, wt[:, kt, dt_ * 128:(dt_ + 1) * 128],
                                     qxt[:, kt, :], start=(kt == 0), stop=(kt == CK - 1))
                qk_evict(psq, qT, dt_, slice(ch * 512, (ch + 1) * 512),
                         rq_bc[:, ch * 512:(ch + 1) * 512],
                         mrq_bc[:, ch * 512:(ch + 1) * 512],
                         s_col[(0, dt_)], b_col[(0, dt_)])

        # -- attention for the 4 heads of this group --
        for hg in range(HG):
            h = g * HG + hg
            rb = (hg % 2) * 64
            for slot in range(2):
                psy_t = psy.tile([128, 512], F32, name="py")
                P_list = []
                for kt in range(16):
                    pss = ps.tile([128, 512], F32, name="ps")
                    nc.tensor.matmul(pss,
                                     kT[:, hg, kt * 128:(kt + 1) * 128],
                                     qT[:, hg, slot * 512:(slot + 1) * 512],
                                     start=True, stop=True)
                    P_t = pp.tile([128, 512], BF16, name="P")
                    nc.scalar.activation(P_t, pss, AF.Exp, scale=1.0 / math.sqrt(HD))
                    msk_sl = (masksb[:, kt, :] if slot == 0
                              else None if kt < 4
                              else masksb[:, kt - 4, :])
                    if msk_sl is not None:
                        nc.vector.tensor_mul(P_t, P_t, msk_sl)
                    P_list.append(P_t)
                for kt in range(16):
                    nc.tensor.matmul(psy_t, va[:, kt, hg, :], P_list[kt],
                                     start=(kt == 0), stop=(kt == 15))
                rec = stg.tile([128, 512], F32, name="rec")
                nc.vector.reciprocal(rec[64:65, :], psy_t[64:65, :])
                recb = stg.tile([128, 512], BF16, name="recb")
                nc.vector.tensor_copy(recb[64:65, :], rec[64:65, :])
                pbc = psb.tile([64, 512], F32, name="pb")
                nc.tensor.matmul(pbc, ones_t[64:65, :], recb[64:65, :],
                                 start=True, stop=True)
                yf = stg.tile([64, 512], F32, name="yf")
                nc.scalar.copy(yf, psy_t[0:64, :])
                yst = stg.tile([64, 512], BF16, name="yst")
                nc.vector.tensor_tensor(yst, yf, pbc, op=OP.mult)
                nc.sync.dma_start(
                    out=yT2[rb:rb + 64, h // 2, slot * 512:(slot + 1) * 512],
                    in_=yst)

    attn_ctx.close()
    attn_ctx0.close()

    # ---- proj + residual ----
    mlp = pool("mlp", 1)
    mstr = pool("mstr", 2)
    wstream = pool("wstream", 3)
    c2 = pool("c2", 1)
    x2 = mlp.tile([128, NQT, C], F32)
    bproj_sb = c2.tile([128, C], F32)
    nc.sync.dma_start(out=bproj_sb, in_=d["bproj_bc"])
    g2sb = c2.tile([128, C], F32)
    nc.sync.dma_start(out=g2sb, in_=d["g2bc"])
    b2sb = c2.tile([128, C], F32)
    nc.sync.dma_start(out=b2sb, in_=d["b2bc"])
    bfc2_sb = c2.tile([128, C], F32)
    nc.sync.dma_start(out=bfc2_sb, in_=d["bfc2_bc"])
    bfc_sb = c2.tile([128, NGT], F32)
    nc.sync.dma_start(out=bfc_sb,
                      in_=d["bfc_col"].rearrange("(k p) o -> p (k o)", p=128))

    wpj = []
    for kt in range(CK):
        w = c2.tile([128, C], BF16, name=f"wpj{kt}")
        nc.sync.dma_start(out=w, in_=d["wproj"][kt * 128:(kt + 1) * 128, :])
        wpj.append(w)

    for m in range(NQT):
        xr_t = mstr.tile([128, C], F32, name="xr_t")
        nc.sync.dma_start(out=xr_t, in_=d["xr"][m * 128:(m + 1) * 128, :])
        for n in range(2):
            psp = ps.tile([128, 512], F32, name="ps")
            for kt in range(CK):
                nc.tensor.matmul(psp, yT2[:, kt, m * 128:(m + 1) * 128],
                                 wpj[kt][:, n * 512:(n + 1) * 512],
                                 start=(kt == 0), stop=(kt == CK - 1))
            sl = slice(n * 512, (n + 1) * 512)
            nc.vector.tensor_tensor(x2[:, m, sl], psp, xr_t[:, sl], op=OP.add)
            nc.vector.tensor_tensor(x2[:, m, sl], x2[:, m, sl], bproj_sb[:, sl],
                                    op=OP.add)

    # ---- LN2 + transpose ----
    hT = mlp.tile([128, CK, TQ], BF16)
    for m in range(NQT):
        st = stats.tile([128, 2, 6], F32, name="st")
        resh = x2[:, m, :].rearrange("p (n f) -> p n f", f=512)
        for i in range(2):
            nc.vector.bn_stats(out=st[:, i, :], in_=resh[:, i, :])
        mv = stats.tile([128, 2], F32, name="mv")
        nc.vector.bn_aggr(out=mv, in_=st)
        sd = stats.tile([128, 1], F32, name="sd")
        nc.scalar.activation(sd, mv[:, 1:2], AF.Sqrt, bias=eps_t)
        r2 = stats.tile([128, 1], F32, name="r2")
        nc.vector.reciprocal(r2, sd)
        hm = mstr.tile([128, C], F32, name="hm")
        nc.vector.tensor_scalar(hm, in0=x2[:, m, :], scalar1=mv[:, 0:1],
                                scalar2=r2, op0=OP.subtract, op1=OP.mult)
        nc.vector.tensor_tensor(hm, hm, g2sb, op=OP.mult)
        hmb = mstr.tile([128, C], BF16, name="hmb")
        nc.vector.tensor_tensor(hmb, hm, b2sb, op=OP.add)
        for ck in range(CK):
            pst = ps.tile([128, 512], F32, name="ps")
            pstv = pst.bitcast(BF16)[:, 0:128]
            nc.tensor.transpose(pstv, hmb[:, ck * 128:(ck + 1) * 128], ident)
            nc.scalar.copy(hT[:, ck, m * 128:(m + 1) * 128], pstv)

    # ---- MLP ----
    hid = mlp.tile([128, NGT, 512], BF16, name="hid")
    psacc = pool("psacc", 4, space="PSUM")
    ostg = pool("ostg", 3)
    for th in range(2):
        for gtg in range(NGT // 4):
            wfcg = wstream.tile([128, CK, 512], BF16, name="wfcg")
            for kt in range(CK):
                nc.sync.dma_start(out=wfcg[:, kt, :],
                                  in_=d["wfc"][kt * 128:(kt + 1) * 128,
                                               gtg * 512:(gtg + 1) * 512])
            for gi in range(4):
                gt = gtg * 4 + gi
                psf = ps.tile([128, 512], F32, name="ps")
                for kt in range(CK):
                    nc.tensor.matmul(psf, wfcg[:, kt, gi * 128:(gi + 1) * 128],
                                     hT[:, kt, th * 512:(th + 1) * 512],
                                     start=(kt == 0), stop=(kt == CK - 1))
                nc.scalar.activation(hid[:, gt, :], psf, AF.Gelu,
                                     bias=bfc_sb[:, gt:gt + 1])
        for n in range(2):
            accs = [psacc.tile([128, 512], F32, name="acc") for _ in range(4)]
            for gkt in range(NGT):
                wf2 = wstream.tile([128, 512], BF16, name="wf2")
                nc.sync.dma_start(out=wf2,
                                  in_=d["wfc2"][gkt * 128:(gkt + 1) * 128,
                                                n * 512:(n + 1) * 512])
                for ml_ in range(4):
                    nc.tensor.matmul(accs[ml_],
                                     hid[:, gkt, ml_ * 128:(ml_ + 1) * 128],
                                     wf2, start=(gkt == 0), stop=(gkt == NGT - 1))
            for ml_ in range(4):
                m = th * 4 + ml_
                osb = ostg.tile([128, 512], F32, name="osb")
                nc.vector.tensor_tensor(osb, accs[ml_], x2[:, m, n * 512:(n + 1) * 512],
                                        op=OP.add)
                nc.vector.tensor_tensor(osb, osb, bfc2_sb[:, n * 512:(n + 1) * 512],
                                        op=OP.add)
                nc.sync.dma_start(out=out[m * 128:(m + 1) * 128, n * 512:(n + 1) * 512],
                                  in_=osb)
